# revision 5
# baseline (speedup 1.0000x reference)
"""Causal self-attention (B=4, T=2048, C=2048, H=16) on 8 NeuronCores.

Sharding: core c = (b, g) with b = c // 2 (batch), g = c % 2 (head group of 8
heads = 1024 channels). Data parallel over B, tensor parallel over heads; the
output projection is computed per head-group and the two partials per batch are
summed on the host (+ bp).

Per-core device program (identical SPMD program, different data):
  phase 1: qT = (Wq_g x_b^T) * 1/sqrt(d) + bq  -> DRAM   [fp32r matmuls]
           kT, v likewise (v stored bf16, natural [t, d] layout)
  phase 2: per head: S = qT^T kT (PSUM), + attn mask (DVE), exp on ACT with
           accum_out = row sum (no max subtraction -- |S'| <~ 15 so exp is
           safe in fp32), normalize by reciprocal, PE-transpose P (bf16),
           y^T = sum_j v_j^T P^T_j (bf16 matmuls) -> DRAM
  phase 3: out_part = y^T^T Wp_g^T  [fp32r matmuls] -> DRAM
"""

import math

import numpy as np

import concourse.bass as bass
import concourse.bacc as bacc
import concourse.mybir as mybir
from concourse import masks
from concourse.tile import TileContext
from concourse.bass_utils import run_bass_kernel_spmd

T = 2048
C = 2048
N_HEAD = 16
D = 128          # head dim
HG = 8           # heads per core
CG = HG * D      # 1024: per-core projection width
B = 4
N_CORES = 8
NEG = -1.0e30

F32 = mybir.dt.float32
F32R = mybir.dt.float32r
BF16 = mybir.dt.bfloat16

_NC_CACHE = None


def _build_program():
    nc = bacc.Bacc("TRN2", target_bir_lowering=False, debug=False)

    xT = nc.dram_tensor("xT", [C, T], F32R, kind="ExternalInput")
    wqT = nc.dram_tensor("wqT", [C, CG], F32R, kind="ExternalInput")
    wkT = nc.dram_tensor("wkT", [C, CG], F32R, kind="ExternalInput")
    wvT = nc.dram_tensor("wvT", [C, CG], F32R, kind="ExternalInput")
    bq = nc.dram_tensor("bq", [128, HG], F32, kind="ExternalInput")
    bk = nc.dram_tensor("bk", [128, HG], F32, kind="ExternalInput")
    bv = nc.dram_tensor("bv", [128, CG], F32, kind="ExternalInput")
    wpT = nc.dram_tensor("wpT", [CG, C], F32R, kind="ExternalInput")
    mask = nc.dram_tensor("mask", [128, T], F32, kind="ExternalInput")
    cd = nc.dram_tensor("cd", [128, 128], F32, kind="ExternalInput")
    out = nc.dram_tensor("out", [T, C], F32, kind="ExternalOutput")

    qTd = nc.dram_tensor("qTd", [CG, T], F32R)
    kTd = nc.dram_tensor("kTd", [CG, T], F32R)
    vd = nc.dram_tensor("vd", [T, CG], BF16)
    yTd = nc.dram_tensor("yTd", [CG, T], F32R)

    add = mybir.AluOpType.add
    Exp = mybir.ActivationFunctionType.Exp
    Copy = mybir.ActivationFunctionType.Copy

    with TileContext(nc) as tc:
        # ---- constants that live for the whole kernel ----
        with tc.tile_pool(name="const", bufs=1) as cpool:
            mask_sb = cpool.tile([128, T], F32)
            nc.sync.dma_start(out=mask_sb, in_=mask[:, :])
            cd_sb = cpool.tile([128, 128], F32)
            nc.sync.dma_start(out=cd_sb, in_=cd[:, :])
            ident = cpool.tile([128, 128], BF16)
            masks.make_identity(nc, ident[:])

            # ================= phase 1: QKV projections =================
            with (
                tc.tile_pool(name="p1x", bufs=1) as xpool,
                tc.tile_pool(name="p1w", bufs=2) as wpool,
                tc.tile_pool(name="p1b", bufs=1) as bpool,
                tc.tile_pool(name="p1ps", bufs=4, space="PSUM") as pspool,
                tc.tile_pool(name="p1o", bufs=4) as opool,
            ):
                xt = xpool.tile([128, 16, T], F32R)
                for cg in range(4):
                    nc.sync.dma_start(
                        out=xt[:, cg * 4:(cg + 1) * 4, :],
                        in_=xT[cg * 512:(cg + 1) * 512, :].rearrange(
                            "(cc p) t -> p cc t", p=128
                        ),
                    )
                bq_sb = bpool.tile([128, HG], F32)
                nc.sync.dma_start(out=bq_sb, in_=bq[:, :])
                bk_sb = bpool.tile([128, HG], F32)
                nc.sync.dma_start(out=bk_sb, in_=bk[:, :])
                bv_sb = bpool.tile([128, CG], F32)
                nc.sync.dma_start(out=bv_sb, in_=bv[:, :])

                # q and k: out layout [d, t] (chunks of 128 d-rows)
                for w_dram, b_sb, o_dram in (
                    (wqT, bq_sb, qTd),
                    (wkT, bk_sb, kTd),
                ):
                    for dc in range(HG):
                        wt = wpool.tile([128, 16, 128], F32R, tag="wqk")
                        nc.sync.dma_start(
                            out=wt,
                            in_=w_dram[:, dc * 128:(dc + 1) * 128].rearrange(
                                "(cc p) d -> p cc d", p=128
                            ),
                        )
                        for tr in range(4):
                            ps = pspool.tile([128, 512], F32, tag="ps1")
                            for cc in range(16):
                                nc.tensor.matmul(
                                    ps,
                                    wt[:, cc, :],
                                    xt[:, cc, tr * 512:(tr + 1) * 512],
                                    start=(cc == 0),
                                    stop=(cc == 15),
                                )
                            ob = opool.tile([128, 512], F32R, tag="o1")
                            nc.vector.tensor_scalar_add(ob, ps, b_sb[:, dc:dc + 1])
                            nc.sync.dma_start(
                                out=o_dram[dc * 128:(dc + 1) * 128,
                                           tr * 512:(tr + 1) * 512],
                                in_=ob,
                            )

                # v: natural layout [t, d], bf16
                for dr in range(4):
                    wv_t = wpool.tile([128, 16, 256], F32R, tag="wv")
                    nc.sync.dma_start(
                        out=wv_t,
                        in_=wvT[:, dr * 256:(dr + 1) * 256].rearrange(
                            "(cc p) d -> p cc d", p=128
                        ),
                    )
                    for tcb in range(16):
                        ps = pspool.tile([128, 256], F32, tag="psv")
                        for cc in range(16):
                            nc.tensor.matmul(
                                ps,
                                xt[:, cc, tcb * 128:(tcb + 1) * 128],
                                wv_t[:, cc, :],
                                start=(cc == 0),
                                stop=(cc == 15),
                            )
                        vb = opool.tile([128, 256], BF16, tag="ov")
                        nc.vector.tensor_tensor(
                            vb, ps, bv_sb[:, dr * 256:(dr + 1) * 256], add
                        )
                        nc.sync.dma_start(
                            out=vd[tcb * 128:(tcb + 1) * 128,
                                   dr * 256:(dr + 1) * 256],
                            in_=vb,
                        )

            # ================= phase 2: attention per head =================
            with (
                tc.tile_pool(name="p2qkv", bufs=2) as qkvp,
                tc.tile_pool(name="p2s", bufs=4) as spool,
                tc.tile_pool(name="p2p", bufs=4) as ppool,
                tc.tile_pool(name="p2pt", bufs=2) as ptpool,
                tc.tile_pool(name="p2stat", bufs=8) as stpool,
                tc.tile_pool(name="p2y", bufs=2) as ypool,
                tc.tile_pool(name="p2psqk", bufs=3, space="PSUM") as psqk,
                tc.tile_pool(name="p2pst", bufs=2, space="PSUM") as pst,
                tc.tile_pool(name="p2psy", bufs=2, space="PSUM") as psy,
            ):
                for h in range(HG):
                    qh = qkvp.tile([128, T], F32R, tag="qh")
                    nc.sync.dma_start(out=qh, in_=qTd[h * 128:(h + 1) * 128, :])
                    kh = qkvp.tile([128, T], F32R, tag="kh")
                    nc.sync.dma_start(out=kh, in_=kTd[h * 128:(h + 1) * 128, :])
                    vh = qkvp.tile([128, 16, 128], BF16, tag="vh")
                    nc.sync.dma_start(
                        out=vh,
                        in_=vd[:, h * 128:(h + 1) * 128].rearrange(
                            "(tc p) d -> p tc d", p=128
                        ),
                    )
                    for g in range(4):  # groups of 4 i-blocks (i512)
                        njc = 4 * (g + 1)
                        pts = ptpool.tile([128, 16, 512], BF16, tag="pt")
                        # zero the above-diagonal corner blocks
                        for jc in range(4 * g + 1, njc):
                            nc.vector.memset(
                                pts[:, jc, 0:(jc - 4 * g) * 128], 0.0
                            )
                        for s in range(4):
                            ib = 4 * g + s
                            jlen = (ib + 1) * 128
                            ssb = spool.tile([128, 2048], F32, tag="ssb")
                            j0 = 0
                            while j0 < jlen:
                                w = min(512, jlen - j0)
                                ps = psqk.tile([128, 512], F32, tag="pqk")
                                nc.tensor.matmul(
                                    ps[:, :w],
                                    qh[:, ib * 128:(ib + 1) * 128],
                                    kh[:, j0:j0 + w],
                                    start=True,
                                    stop=True,
                                )
                                nc.vector.tensor_tensor(
                                    ssb[:, j0:j0 + w], ps[:, :w],
                                    mask_sb[:, j0:j0 + w], add,
                                )
                                j0 += w
                            # causal mask on the diagonal 128 cols
                            nc.vector.tensor_tensor(
                                ssb[:, jlen - 128:jlen],
                                ssb[:, jlen - 128:jlen], cd_sb, add,
                            )
                            pb = ppool.tile([128, 2048], BF16, tag="pb")
                            sacc = stpool.tile([128, 1], F32, tag="sacc")
                            nc.scalar.activation(
                                pb[:, :jlen], ssb[:, :jlen], Exp, accum_out=sacc
                            )
                            rr = stpool.tile([128, 1], F32, tag="rr")
                            nc.vector.reciprocal(rr, sacc)
                            pn = ppool.tile([128, 2048], BF16, tag="pn")
                            nc.vector.tensor_scalar_mul(
                                pn[:, :jlen], pb[:, :jlen], rr
                            )
                            # transpose P blocks -> pts[:, jc, s*128:(s+1)*128]
                            for jg in range(0, ib + 1, 4):
                                n = min(4, ib + 1 - jg)
                                tps = pst.tile([128, 4, 128], BF16, tag="tps")
                                for q in range(n):
                                    nc.tensor.transpose(
                                        tps[:, q, :],
                                        pn[:, (jg + q) * 128:(jg + q + 1) * 128],
                                        ident,
                                    )
                                nc.vector.tensor_copy(
                                    pts[:, jg:jg + n, s * 128:(s + 1) * 128],
                                    tps[:, :n, :],
                                )
                        # AV for the whole i512 group
                        yt = psy.tile([128, 512], F32, tag="yt")
                        for jc in range(njc):
                            nc.tensor.matmul(
                                yt,
                                vh[:, jc, :],
                                pts[:, jc, :],
                                start=(jc == 0),
                                stop=(jc == njc - 1),
                            )
                        ysb = ypool.tile([128, 512], F32R, tag="ysb")
                        nc.scalar.activation(ysb, yt, Copy)
                        nc.sync.dma_start(
                            out=yTd[h * 128:(h + 1) * 128,
                                    g * 512:(g + 1) * 512],
                            in_=ysb,
                        )

            # ================= phase 3: output projection =================
            with (
                tc.tile_pool(name="p3w", bufs=1) as wp3,
                tc.tile_pool(name="p3y", bufs=2) as yp3,
                tc.tile_pool(name="p3ps", bufs=4, space="PSUM") as ps3,
                tc.tile_pool(name="p3o", bufs=4) as op3,
            ):
                wp_sb = wp3.tile([128, HG, C], F32R)
                nc.sync.dma_start(
                    out=wp_sb,
                    in_=wpT.rearrange("(h p) c -> p h c", p=128),
                )
                for tcb in range(16):
                    yb = yp3.tile([128, HG, 128], F32R, tag="yb")
                    nc.sync.dma_start(
                        out=yb,
                        in_=yTd[:, tcb * 128:(tcb + 1) * 128].rearrange(
                            "(h p) t -> p h t", p=128
                        ),
                    )
                    for cr in range(4):
                        ps = ps3.tile([128, 512], F32, tag="ps3")
                        for h in range(HG):
                            nc.tensor.matmul(
                                ps,
                                yb[:, h, :],
                                wp_sb[:, h, cr * 512:(cr + 1) * 512],
                                start=(h == 0),
                                stop=(h == HG - 1),
                            )
                        ob = op3.tile([128, 512], F32, tag="ob")
                        nc.scalar.activation(ob, ps, Copy)
                        nc.sync.dma_start(
                            out=out[tcb * 128:(tcb + 1) * 128,
                                    cr * 512:(cr + 1) * 512],
                            in_=ob,
                        )
    nc.compile()
    return nc


def get_nc():
    global _NC_CACHE
    if _NC_CACHE is None:
        _NC_CACHE = _build_program()
    return _NC_CACHE


def prep_core_inputs(inputs):
    """Host-side sharding / layout prep: slice per (b, g), transpose to the
    layouts the device program wants, fold the 1/sqrt(d) softmax scale into
    Wq/bq."""
    f = lambda a: np.asarray(a, dtype=np.float32)
    x = f(inputs["x"])
    am = f(inputs["attn_mask"])
    Wq, bq_ = f(inputs["Wq"]), f(inputs["bq"])
    Wk, bk_ = f(inputs["Wk"]), f(inputs["bk"])
    Wv, bv_ = f(inputs["Wv"]), f(inputs["bv"])
    Wp = f(inputs["Wp"])
    scale = 1.0 / math.sqrt(D)

    ii = np.arange(128)
    cd_t = np.where(ii[None, :] <= ii[:, None], 0.0, NEG).astype(np.float32)

    per_g = []
    for g in range(2):
        sl = slice(g * CG, (g + 1) * CG)
        per_g.append(dict(
            wqT=np.ascontiguousarray(Wq[sl].T) * scale,
            wkT=np.ascontiguousarray(Wk[sl].T),
            wvT=np.ascontiguousarray(Wv[sl].T),
            bq=np.ascontiguousarray((bq_[sl] * scale).reshape(HG, 128).T),
            bk=np.ascontiguousarray(bk_[sl].reshape(HG, 128).T),
            bv=np.ascontiguousarray(np.broadcast_to(bv_[sl], (128, CG))),
            wpT=np.ascontiguousarray(Wp[:, sl].T),
        ))

    in_maps = []
    for core in range(N_CORES):
        b, g = core // 2, core % 2
        m = dict(per_g[g])
        m["xT"] = np.ascontiguousarray(x[b].T)
        m["mask"] = np.ascontiguousarray(
            np.broadcast_to(am[b, 0, 0, :], (128, T))
        )
        m["cd"] = cd_t
        in_maps.append(m)
    return in_maps


def run(inputs, trace=False):
    nc = get_nc()
    in_maps = prep_core_inputs(inputs)
    rr = run_bass_kernel_spmd(nc, in_maps, list(range(N_CORES)), trace=trace)
    bp = np.asarray(inputs["bp"], dtype=np.float32)
    y = np.empty((B, T, C), dtype=np.float32)
    for b in range(B):
        y[b] = rr.results[2 * b]["out"] + rr.results[2 * b + 1]["out"] + bp[None, :]
    return y, rr


def kernel(**inputs):
    y, _ = run(inputs)
    return y
